# revision 24
# baseline (speedup 1.0000x reference)
"""Trainium2 Bass kernel for nn_Attender_20263655702790.

Computation (reference):
  recal  = softmax(local_landmarks, axis=K)          # [N,B,K,H,W], K=13
  pooled = einsum("nbkhw,nbchw->bkc", recal, gmaps)  # [B,K,C]
  out    = pooled / max(||pooled||_2 over C, 1e-12)  # [B,K,C]

Shapes: N=2, B=32, K=13, C=512, H=W=28 (HW=784). All float32.

Sharding: data-parallel over B across 8 NeuronCores (4 batches per core).
Per-core DRAM inputs (prepared host-side in kernel()):
  lmk  [104, 784]        natural-layout landmarks, row = (n, b_local, k)
  gmap [2, 4, 784, 512]  global maps pre-transposed host-side to hw-major
Per-core output:
  out  [4, 13, 512]

Device algorithm per core:
  - DMA lmk; PE-transpose each [104, 112] hw-slab -> [112, 104] (hw on
    partitions, (n,b,k) on free).
  - Softmax over K without max-subtraction (inputs are unit normals, exp is
    safely in range): exp on ACT, 13-wide block sums on DVE, reciprocal,
    broadcast multiply.
  - For each local b: stream both n-slabs of gmap ([112, 7, 512] tiles),
    accumulate pooled[13, 512] in PSUM over 14 matmuls
    (lhsT = recal [112hw, 13k], rhs = gmap [112hw, 512c]).
  - L2 normalize over C on ACT/DVE, single output DMA.

The kernel is memory-bound: ~13.3 MB HBM traffic per core.
"""

import json

import numpy as np

import concourse.bass as bass
import concourse.tile as tile
from concourse import mybir
from concourse.masks import make_identity

F32 = mybir.dt.float32
AF = mybir.ActivationFunctionType

N = 2
B = 32
K = 13
C = 512
HW = 784
N_CORES = 8
BL = B // N_CORES  # 4 local batches per core
NB = N * BL  # 8 (n, b_local) pairs
P = 112  # hw tile partition count; 784 = 7 * 112
T = HW // P  # 7 hw tiles
EPS = 1e-12

_PATCHED = False


def _split_sync_waits(bir_bytes):
    """The pinned walrus build rejects instructions carrying more than one
    sem-wait (setupSyncWait: "Too many sync wait commands"). Tile sometimes
    attaches several waits to one instruction (matmuls with multiple
    producers, the kernel-tail drain). Hoist the extra waits onto injected
    same-engine NoOps immediately before the instruction — identical
    semantics, one wait per instruction."""
    m = json.loads(bir_bytes)
    counter = 0
    for fn in m.get("functions", []):
        for blk in fn.get("blocks", []):
            insts = blk.get("instructions")
            if not insts:
                continue
            out = []
            for inst in insts:
                si = inst.get("sync_info")
                if si:
                    waits = si.get("on_wait") or []
                    if len(waits) > 1:
                        for w in waits[:-1]:
                            counter += 1
                            out.append(
                                {
                                    "debug": inst.get("debug", 0),
                                    "engine": inst["engine"],
                                    "ins": [],
                                    "outs": [],
                                    "name": f"{inst['name']}-sw{counter}",
                                    "opcode": "NoOp",
                                    "sync_info": {
                                        "on_wait": [w],
                                        "on_update": [],
                                    },
                                }
                            )
                        si["on_wait"] = [waits[-1]]
                out.append(inst)
            blk["instructions"] = out
    return json.dumps(m).encode()


def _patch_compile():
    """Route every BIR compile through _split_sync_waits."""
    global _PATCHED
    if _PATCHED:
        return
    _PATCHED = True
    import concourse.bass_utils as bu
    import concourse.bass2jax as b2j

    orig = bu.compile_bir_kernel

    def patched(bir_json, tmpdir, neff_name="file.neff"):
        return orig(_split_sync_waits(bir_json), tmpdir, neff_name)

    bu.compile_bir_kernel = patched
    b2j.compile_bir_kernel = patched


def build_bass(mm_dtype="fp32"):
    """Build the per-core Bass module. mm_dtype: fp32 | fp32r | fp16x2.

    fp32   — exact, but PE runs fp32 matmuls at 4 cycles/row.
    fp32r  — 1 cycle/row, TF32-like precision (~2e-4 rel err).
    fp16x2 — g split host-side into fp16 hi+lo planes (same total bytes as
             f32), recal split on-chip; 2 streaming passes at 1 cycle/row
             reach ~fp32 precision: pooled = [rhi|rlo]@ghi + rhi@glo, all
             accumulated in one PSUM group (dropped rlo*glo term ~2^-22).
    """
    _patch_compile()
    nc = bass.Bass()
    F16 = mybir.dt.float16
    split16 = mm_dtype == "fp16x2"
    # float32r is bit-identical to f32 in memory; typing the operand chain as
    # f32r lets the PE run matmuls at 1 cycle/row instead of fp32's 4.
    MD = mybir.dt.float32r if mm_dtype == "fp32r" else F32
    lmk = nc.dram_tensor("lmk", [NB * K, HW], F32, kind="ExternalInput")
    # gmap host layout: hw split as (t p), stored [n, b, p, t, c] so each SBUF
    # partition row is one contiguous run (T*C elements) per (n, b) DMA.
    if split16:
        # hi and lo planes packed per partition row: one 14 KB contiguous run
        g2_d = nc.dram_tensor("g2", [N, BL, P, 2, T, C], F16, kind="ExternalInput")
    else:
        gmap = nc.dram_tensor("gmap", [N, BL, P, T, C], MD, kind="ExternalInput")
    out_d = nc.dram_tensor("out", [BL, K, C], F32, kind="ExternalOutput")

    with tile.TileContext(nc) as tc:
        with (
            tc.tile_pool(name="consts", bufs=1) as consts,
            tc.tile_pool(name="lmp", bufs=1) as lmp,
            tc.tile_pool(name="recp", bufs=1) as recp,
            tc.tile_pool(name="gp", bufs=4) as gp,
            tc.tile_pool(name="small", bufs=8) as small,
            tc.tile_pool(name="outp", bufs=1) as outp,
            tc.tile_pool(name="ptp", bufs=2, space="PSUM") as ptp,
            tc.tile_pool(name="pmp", bufs=4, space="PSUM") as pmp,
        ):
            # identity as NEFF-embedded const: gpsimd (make_identity's engine)
            # starts ~6us late and would gate the transpose chain
            ident_d = nc.inline_tensor(np.eye(NB * K, dtype=np.float32), "ident")
            ident = consts.tile([NB * K, NB * K], F32)
            nc.sync.dma_start(out=ident, in_=ident_d[:, :])
            sb_eps = consts.tile([K, 1], F32)
            nc.vector.memset(sb_eps, EPS)

            sb_lmk = lmp.tile([NB * K, HW], F32)
            nc.sync.dma_start(out=sb_lmk, in_=lmk[:, :])

            # recal[p_hw, t, (n b k)] after softmax over k
            sb_rec = recp.tile([P, T, NB * K], MD)
            sb_rcp = recp.tile([P, T, NB], F32)
            if split16:
                # hi/lo packed per nb with lo at column 32 so the PSUM lo rows
                # start at partition 32 (engine partition base must be 0/32/64/96);
                # columns 13..31 stay zero.
                sb_rhl = recp.tile([P, T, NB, 45], F16)
                nc.vector.memset(sb_rhl[:, :, :, K:32], 0.0)

            for t in range(T):
                pt = ptp.tile([P, NB * K], F32, tag="pt")
                nc.tensor.transpose(pt, sb_lmk[:, t * P : (t + 1) * P], ident)
                rec_t = sb_rec[:, t, :]
                nc.scalar.activation(rec_t, pt, AF.Exp)
                rec3 = rec_t.rearrange("p (nb k) -> p nb k", k=K)
                ssum = small.tile([P, NB], F32, tag="ssum")
                nc.vector.reduce_sum(ssum, rec3, axis=mybir.AxisListType.X)
                nc.vector.reciprocal(sb_rcp[:, t, :], ssum)
                nc.vector.tensor_mul(
                    rec3,
                    rec3,
                    sb_rcp[:, t, :].unsqueeze(2).broadcast_to([P, NB, K]),
                )
                if split16:
                    rhl4 = sb_rhl[:, t]
                    nc.vector.tensor_copy(rhl4[:, :, 0:K], rec3)
                    nc.vector.tensor_sub(rhl4[:, :, 32 : 32 + K], rec3, rhl4[:, :, 0:K])

            sb_out = outp.tile([K, BL, C], F32)
            for b in range(BL):
                if split16:
                    ghi_tiles, glo_tiles = [], []
                    for n in range(N):
                        sb_g2 = gp.tile([P, 2, T, C], F16, tag="g")
                        nc.sync.dma_start(out=sb_g2, in_=g2_d[n, b])
                        ghi_tiles.append(sb_g2[:, 0])
                        glo_tiles.append(sb_g2[:, 1])
                    pm = pmp.tile([32 + K, C], F32, tag="pm")
                    for n in range(N):
                        nb = n * BL + b
                        for t in range(T):
                            nc.tensor.matmul(
                                pm,
                                sb_rhl[:, t, nb, :],
                                ghi_tiles[n][:, t, :],
                                start=(n == 0 and t == 0),
                                stop=False,
                            )
                    for n in range(N):
                        nb = n * BL + b
                        for t in range(T):
                            nc.tensor.matmul(
                                pm[0:K, :],
                                sb_rhl[:, t, nb, 0:K],
                                glo_tiles[n][:, t, :],
                                start=False,
                                stop=(n == N - 1 and t == T - 1),
                            )
                    # pooled = pm[hi rows] + pm[lo-correction rows]
                    tmp = small.tile([K, C], F32, tag="tmp")
                    nc.scalar.copy(tmp, pm[32 : 32 + K, :])
                    pooled = small.tile([K, C], F32, tag="pooled")
                    nc.vector.tensor_add(pooled, tmp, pm[0:K, :])
                else:
                    g_tiles = []
                    for n in range(N):
                        sb_g = gp.tile([P, T, C], MD, tag="g")
                        nc.sync.dma_start(out=sb_g, in_=gmap[n, b])
                        g_tiles.append(sb_g)
                    pm = pmp.tile([K, C], F32, tag="pm")
                    for n in range(N):
                        nb = n * BL + b
                        for t in range(T):
                            nc.tensor.matmul(
                                pm,
                                sb_rec[:, t, nb * K : (nb + 1) * K],
                                g_tiles[n][:, t, :],
                                start=(n == 0 and t == 0),
                                stop=(n == N - 1 and t == T - 1),
                            )
                    pooled = pm
                # L2 normalize over C
                sq = small.tile([K, C], F32, tag="sq")
                ss = small.tile([K, 1], F32, tag="ss")
                nc.scalar.activation(sq, pooled, AF.Square, accum_out=ss)
                nrm = small.tile([K, 1], F32, tag="nrm")
                nc.scalar.activation(nrm, ss, AF.Sqrt)
                nc.vector.tensor_max(nrm, nrm, sb_eps)
                rcpn = small.tile([K, 1], F32, tag="rcpn")
                nc.vector.reciprocal(rcpn, nrm)
                nc.vector.tensor_scalar_mul(sb_out[:, b, :], in0=pooled, scalar1=rcpn)
                # SWDGE: keeps the HWDGE prefetch ring free of dependent DMAs
                nc.gpsimd.dma_start(out=out_d[b], in_=sb_out[:, b, :])
    return nc


def _prep_in_maps(local_landmarks, global_maps, mm_dtype="fp32"):
    l = np.ascontiguousarray(np.asarray(local_landmarks, dtype=np.float32)).reshape(
        N, B, K, HW
    )
    g = np.asarray(global_maps, dtype=np.float32).reshape(N, B, C, T, P)
    # [N, B, P, T, C]: hw = t*P + p; partition rows contiguous per (n, b).
    gt = np.ascontiguousarray(g.transpose(0, 1, 4, 3, 2))
    if mm_dtype == "fp16x2":
        ghi = gt.astype(np.float16)
        glo = (gt - ghi.astype(np.float32)).astype(np.float16)
        # [N, B, P, 2, T, C]
        g2 = np.stack([ghi, glo], axis=3)
    in_maps = []
    for c in range(N_CORES):
        bs = slice(BL * c, BL * (c + 1))
        m = {"lmk": np.ascontiguousarray(l[:, bs]).reshape(NB * K, HW)}
        if mm_dtype == "fp16x2":
            m["g2"] = np.ascontiguousarray(g2[:, bs])
        else:
            m["gmap"] = np.ascontiguousarray(gt[:, bs])
        in_maps.append(m)
    return in_maps


def run_on_cores(local_landmarks, global_maps, trace=False, mm_dtype="fp32"):
    """Returns (full_output [32,13,512], BassKernelResults)."""
    from concourse.bass_utils import run_bass_kernel_spmd

    nc = build_bass(mm_dtype=mm_dtype)
    in_maps = _prep_in_maps(local_landmarks, global_maps, mm_dtype=mm_dtype)
    res = run_bass_kernel_spmd(
        nc, in_maps, core_ids=list(range(N_CORES)), trace=trace
    )
    out = np.concatenate([r["out"] for r in res.results], axis=0)
    return np.ascontiguousarray(out, dtype=np.float32), res


def kernel(local_landmarks, global_maps):
    out, _ = run_on_cores(local_landmarks, global_maps, trace=False)
    return out


# revision 25
# speedup vs baseline: 1.3891x; 1.3891x over previous
"""Trainium2 Bass kernel for nn_Attender_20263655702790.

Computation (reference):
  recal  = softmax(local_landmarks, axis=K)          # [N,B,K,H,W], K=13
  pooled = einsum("nbkhw,nbchw->bkc", recal, gmaps)  # [B,K,C]
  out    = pooled / max(||pooled||_2 over C, 1e-12)  # [B,K,C]

Shapes: N=2, B=32, K=13, C=512, H=W=28 (HW=784). All float32.

Sharding: data-parallel over B across 8 NeuronCores (4 batches per core).
Per-core DRAM inputs (prepared host-side in kernel()):
  lmk  [104, 784]        natural-layout landmarks, row = (n, b_local, k)
  gmap [2, 4, 784, 512]  global maps pre-transposed host-side to hw-major
Per-core output:
  out  [4, 13, 512]

Device algorithm per core:
  - DMA lmk; PE-transpose each [104, 112] hw-slab -> [112, 104] (hw on
    partitions, (n,b,k) on free).
  - Softmax over K without max-subtraction (inputs are unit normals, exp is
    safely in range): exp on ACT, 13-wide block sums on DVE, reciprocal,
    broadcast multiply.
  - For each local b: stream both n-slabs of gmap ([112, 7, 512] tiles),
    accumulate pooled[13, 512] in PSUM over 14 matmuls
    (lhsT = recal [112hw, 13k], rhs = gmap [112hw, 512c]).
  - L2 normalize over C on ACT/DVE, single output DMA.

The kernel is memory-bound: ~13.3 MB HBM traffic per core.
"""

import json

import numpy as np

import concourse.bass as bass
import concourse.tile as tile
from concourse import mybir
from concourse.masks import make_identity

F32 = mybir.dt.float32
AF = mybir.ActivationFunctionType

N = 2
B = 32
K = 13
C = 512
HW = 784
N_CORES = 8
BL = B // N_CORES  # 4 local batches per core
NB = N * BL  # 8 (n, b_local) pairs
P = 112  # hw tile partition count; 784 = 7 * 112
T = HW // P  # 7 hw tiles
EPS = 1e-12

_PATCHED = False


def _split_sync_waits(bir_bytes):
    """The pinned walrus build rejects instructions carrying more than one
    sem-wait (setupSyncWait: "Too many sync wait commands"). Tile sometimes
    attaches several waits to one instruction (matmuls with multiple
    producers, the kernel-tail drain). Hoist the extra waits onto injected
    same-engine NoOps immediately before the instruction — identical
    semantics, one wait per instruction."""
    m = json.loads(bir_bytes)
    counter = 0
    for fn in m.get("functions", []):
        for blk in fn.get("blocks", []):
            insts = blk.get("instructions")
            if not insts:
                continue
            out = []
            for inst in insts:
                si = inst.get("sync_info")
                if si:
                    waits = si.get("on_wait") or []
                    if len(waits) > 1:
                        for w in waits[:-1]:
                            counter += 1
                            out.append(
                                {
                                    "debug": inst.get("debug", 0),
                                    "engine": inst["engine"],
                                    "ins": [],
                                    "outs": [],
                                    "name": f"{inst['name']}-sw{counter}",
                                    "opcode": "NoOp",
                                    "sync_info": {
                                        "on_wait": [w],
                                        "on_update": [],
                                    },
                                }
                            )
                        si["on_wait"] = [waits[-1]]
                out.append(inst)
            blk["instructions"] = out
    return json.dumps(m).encode()


def _patch_compile():
    """Route every BIR compile through _split_sync_waits."""
    global _PATCHED
    if _PATCHED:
        return
    _PATCHED = True
    import concourse.bass_utils as bu
    import concourse.bass2jax as b2j

    orig = bu.compile_bir_kernel

    def patched(bir_json, tmpdir, neff_name="file.neff"):
        return orig(_split_sync_waits(bir_json), tmpdir, neff_name)

    bu.compile_bir_kernel = patched
    b2j.compile_bir_kernel = patched


def build_bass(mm_dtype="fp32"):
    """Build the per-core Bass module. mm_dtype: fp32 | fp32r | fp16x2.

    fp32   — exact, but PE runs fp32 matmuls at 4 cycles/row.
    fp32r  — 1 cycle/row, TF32-like precision (~2e-4 rel err).
    fp16x2 — g split host-side into fp16 hi+lo planes (same total bytes as
             f32), recal split on-chip; 2 streaming passes at 1 cycle/row
             reach ~fp32 precision: pooled = [rhi|rlo]@ghi + rhi@glo, all
             accumulated in one PSUM group (dropped rlo*glo term ~2^-22).
    """
    _patch_compile()
    nc = bass.Bass()
    F16 = mybir.dt.float16
    split16 = mm_dtype == "fp16x2"
    half16 = mm_dtype == "fp16"
    # float32r is bit-identical to f32 in memory; typing the operand chain as
    # f32r lets the PE run matmuls at 1 cycle/row instead of fp32's 4.
    MD = mybir.dt.float32r if mm_dtype == "fp32r" else F32
    lmk = nc.dram_tensor("lmk", [NB * K, HW], F32, kind="ExternalInput")
    # gmap host layout: hw split as (t p), stored [n, b, p, t, c] so each SBUF
    # partition row is one contiguous run (T*C elements) per (n, b) DMA.
    if split16:
        # hi and lo planes packed per partition row: one 14 KB contiguous run
        g2_d = nc.dram_tensor("g2", [N, BL, P, 2, T, C], F16, kind="ExternalInput")
    elif half16:
        gh_d = nc.dram_tensor("gh", [N, BL, P, T, C], F16, kind="ExternalInput")
    else:
        gmap = nc.dram_tensor("gmap", [N, BL, P, T, C], MD, kind="ExternalInput")
    out_d = nc.dram_tensor("out", [BL, K, C], F32, kind="ExternalOutput")

    with tile.TileContext(nc) as tc:
        with (
            tc.tile_pool(name="consts", bufs=1) as consts,
            tc.tile_pool(name="lmp", bufs=1) as lmp,
            tc.tile_pool(name="recp", bufs=1) as recp,
            tc.tile_pool(name="gp", bufs=4) as gp,
            tc.tile_pool(name="small", bufs=8) as small,
            tc.tile_pool(name="outp", bufs=1) as outp,
            tc.tile_pool(name="ptp", bufs=2, space="PSUM") as ptp,
            tc.tile_pool(name="pmp", bufs=4, space="PSUM") as pmp,
        ):
            # identity as NEFF-embedded const: gpsimd (make_identity's engine)
            # starts ~6us late and would gate the transpose chain
            ident_d = nc.inline_tensor(np.eye(NB * K, dtype=np.float32), "ident")
            ident = consts.tile([NB * K, NB * K], F32)
            nc.sync.dma_start(out=ident, in_=ident_d[:, :])
            sb_eps = consts.tile([K, 1], F32)
            nc.vector.memset(sb_eps, EPS)

            sb_lmk = lmp.tile([NB * K, HW], F32)
            nc.sync.dma_start(out=sb_lmk, in_=lmk[:, :])

            # recal[p_hw, t, (n b k)] after softmax over k
            sb_rec = recp.tile([P, T, NB * K], MD)
            sb_rcp = recp.tile([P, T, NB], F32)
            if split16:
                # hi/lo packed per nb with lo at column 32 so the PSUM lo rows
                # start at partition 32 (engine partition base must be 0/32/64/96);
                # columns 13..31 stay zero.
                sb_rhl = recp.tile([P, T, NB, 45], F16)
                nc.vector.memset(sb_rhl[:, :, :, K:32], 0.0)
            elif half16:
                sb_rh = recp.tile([P, T, NB * K], F16)

            for t in range(T):
                pt = ptp.tile([P, NB * K], F32, tag="pt")
                nc.tensor.transpose(pt, sb_lmk[:, t * P : (t + 1) * P], ident)
                rec_t = sb_rec[:, t, :]
                nc.scalar.activation(rec_t, pt, AF.Exp)
                rec3 = rec_t.rearrange("p (nb k) -> p nb k", k=K)
                ssum = small.tile([P, NB], F32, tag="ssum")
                nc.vector.reduce_sum(ssum, rec3, axis=mybir.AxisListType.X)
                nc.vector.reciprocal(sb_rcp[:, t, :], ssum)
                nc.vector.tensor_mul(
                    rec3,
                    rec3,
                    sb_rcp[:, t, :].unsqueeze(2).broadcast_to([P, NB, K]),
                )
                if split16:
                    rhl4 = sb_rhl[:, t]
                    nc.vector.tensor_copy(rhl4[:, :, 0:K], rec3)
                    nc.vector.tensor_sub(rhl4[:, :, 32 : 32 + K], rec3, rhl4[:, :, 0:K])
                elif half16:
                    nc.vector.tensor_copy(sb_rh[:, t, :], rec_t)

            sb_out = outp.tile([K, BL, C], F32)
            for b in range(BL):
                if split16:
                    ghi_tiles, glo_tiles = [], []
                    for n in range(N):
                        sb_g2 = gp.tile([P, 2, T, C], F16, tag="g")
                        nc.sync.dma_start(out=sb_g2, in_=g2_d[n, b])
                        ghi_tiles.append(sb_g2[:, 0])
                        glo_tiles.append(sb_g2[:, 1])
                    pm = pmp.tile([32 + K, C], F32, tag="pm")
                    for n in range(N):
                        nb = n * BL + b
                        for t in range(T):
                            nc.tensor.matmul(
                                pm,
                                sb_rhl[:, t, nb, :],
                                ghi_tiles[n][:, t, :],
                                start=(n == 0 and t == 0),
                                stop=False,
                            )
                    for n in range(N):
                        nb = n * BL + b
                        for t in range(T):
                            nc.tensor.matmul(
                                pm[0:K, :],
                                sb_rhl[:, t, nb, 0:K],
                                glo_tiles[n][:, t, :],
                                start=False,
                                stop=(n == N - 1 and t == T - 1),
                            )
                    # pooled = pm[hi rows] + pm[lo-correction rows]
                    tmp = small.tile([K, C], F32, tag="tmp")
                    nc.scalar.copy(tmp, pm[32 : 32 + K, :])
                    pooled = small.tile([K, C], F32, tag="pooled")
                    nc.vector.tensor_add(pooled, tmp, pm[0:K, :])
                else:
                    g_tiles = []
                    for n in range(N):
                        sb_g = gp.tile([P, T, C], F16 if half16 else MD, tag="g")
                        nc.sync.dma_start(
                            out=sb_g, in_=gh_d[n, b] if half16 else gmap[n, b]
                        )
                        g_tiles.append(sb_g)
                    pm = pmp.tile([K, C], F32, tag="pm")
                    for n in range(N):
                        nb = n * BL + b
                        for t in range(T):
                            nc.tensor.matmul(
                                pm,
                                sb_rh[:, t, nb * K : (nb + 1) * K]
                                if half16
                                else sb_rec[:, t, nb * K : (nb + 1) * K],
                                g_tiles[n][:, t, :],
                                start=(n == 0 and t == 0),
                                stop=(n == N - 1 and t == T - 1),
                            )
                    pooled = pm
                # L2 normalize over C
                sq = small.tile([K, C], F32, tag="sq")
                ss = small.tile([K, 1], F32, tag="ss")
                nc.scalar.activation(sq, pooled, AF.Square, accum_out=ss)
                nrm = small.tile([K, 1], F32, tag="nrm")
                nc.scalar.activation(nrm, ss, AF.Sqrt)
                nc.vector.tensor_max(nrm, nrm, sb_eps)
                rcpn = small.tile([K, 1], F32, tag="rcpn")
                nc.vector.reciprocal(rcpn, nrm)
                nc.vector.tensor_scalar_mul(sb_out[:, b, :], in0=pooled, scalar1=rcpn)
                # SWDGE: keeps the HWDGE prefetch ring free of dependent DMAs
                nc.gpsimd.dma_start(out=out_d[b], in_=sb_out[:, b, :])
    return nc


def _prep_in_maps(local_landmarks, global_maps, mm_dtype="fp32"):
    l = np.ascontiguousarray(np.asarray(local_landmarks, dtype=np.float32)).reshape(
        N, B, K, HW
    )
    g = np.asarray(global_maps, dtype=np.float32).reshape(N, B, C, T, P)
    # [N, B, P, T, C]: hw = t*P + p; partition rows contiguous per (n, b).
    gt = np.ascontiguousarray(g.transpose(0, 1, 4, 3, 2))
    if mm_dtype == "fp16x2":
        ghi = gt.astype(np.float16)
        glo = (gt - ghi.astype(np.float32)).astype(np.float16)
        # [N, B, P, 2, T, C]
        g2 = np.stack([ghi, glo], axis=3)
    elif mm_dtype == "fp16":
        gh = gt.astype(np.float16)
    in_maps = []
    for c in range(N_CORES):
        bs = slice(BL * c, BL * (c + 1))
        m = {"lmk": np.ascontiguousarray(l[:, bs]).reshape(NB * K, HW)}
        if mm_dtype == "fp16x2":
            m["g2"] = np.ascontiguousarray(g2[:, bs])
        elif mm_dtype == "fp16":
            m["gh"] = np.ascontiguousarray(gh[:, bs])
        else:
            m["gmap"] = np.ascontiguousarray(gt[:, bs])
        in_maps.append(m)
    return in_maps


def run_on_cores(local_landmarks, global_maps, trace=False, mm_dtype="fp32"):
    """Returns (full_output [32,13,512], BassKernelResults)."""
    from concourse.bass_utils import run_bass_kernel_spmd

    nc = build_bass(mm_dtype=mm_dtype)
    in_maps = _prep_in_maps(local_landmarks, global_maps, mm_dtype=mm_dtype)
    res = run_bass_kernel_spmd(
        nc, in_maps, core_ids=list(range(N_CORES)), trace=trace
    )
    out = np.concatenate([r["out"] for r in res.results], axis=0)
    return np.ascontiguousarray(out, dtype=np.float32), res


def kernel(local_landmarks, global_maps):
    out, _ = run_on_cores(local_landmarks, global_maps, trace=False)
    return out


# revision 26
# speedup vs baseline: 1.5164x; 1.0917x over previous
"""Trainium2 Bass kernel for nn_Attender_20263655702790.

Computation (reference):
  recal  = softmax(local_landmarks, axis=K)          # [N,B,K,H,W], K=13
  pooled = einsum("nbkhw,nbchw->bkc", recal, gmaps)  # [B,K,C]
  out    = pooled / max(||pooled||_2 over C, 1e-12)  # [B,K,C]

Shapes: N=2, B=32, K=13, C=512, H=W=28 (HW=784). All float32.

Sharding: data-parallel over B across 8 NeuronCores (4 batches per core).
Per-core DRAM inputs (prepared host-side in kernel()):
  lmk  [104, 784]        natural-layout landmarks, row = (n, b_local, k)
  gmap [2, 4, 784, 512]  global maps pre-transposed host-side to hw-major
Per-core output:
  out  [4, 13, 512]

Device algorithm per core:
  - DMA lmk; PE-transpose each [104, 112] hw-slab -> [112, 104] (hw on
    partitions, (n,b,k) on free).
  - Softmax over K without max-subtraction (inputs are unit normals, exp is
    safely in range): exp on ACT, 13-wide block sums on DVE, reciprocal,
    broadcast multiply.
  - For each local b: stream both n-slabs of gmap ([112, 7, 512] tiles),
    accumulate pooled[13, 512] in PSUM over 14 matmuls
    (lhsT = recal [112hw, 13k], rhs = gmap [112hw, 512c]).
  - L2 normalize over C on ACT/DVE, single output DMA.

The kernel is memory-bound: ~13.3 MB HBM traffic per core.
"""

import json

import numpy as np

import concourse.bass as bass
import concourse.tile as tile
from concourse import mybir
from concourse.masks import make_identity

F32 = mybir.dt.float32
AF = mybir.ActivationFunctionType

N = 2
B = 32
K = 13
C = 512
HW = 784
N_CORES = 8
BL = B // N_CORES  # 4 local batches per core
NB = N * BL  # 8 (n, b_local) pairs
P = 112  # hw tile partition count; 784 = 7 * 112
T = HW // P  # 7 hw tiles
EPS = 1e-12

_PATCHED = False


def _split_sync_waits(bir_bytes):
    """The pinned walrus build rejects instructions carrying more than one
    sem-wait (setupSyncWait: "Too many sync wait commands"). Tile sometimes
    attaches several waits to one instruction (matmuls with multiple
    producers, the kernel-tail drain). Hoist the extra waits onto injected
    same-engine NoOps immediately before the instruction — identical
    semantics, one wait per instruction."""
    m = json.loads(bir_bytes)
    counter = 0
    for fn in m.get("functions", []):
        for blk in fn.get("blocks", []):
            insts = blk.get("instructions")
            if not insts:
                continue
            out = []
            for inst in insts:
                si = inst.get("sync_info")
                if si:
                    waits = si.get("on_wait") or []
                    if len(waits) > 1:
                        for w in waits[:-1]:
                            counter += 1
                            out.append(
                                {
                                    "debug": inst.get("debug", 0),
                                    "engine": inst["engine"],
                                    "ins": [],
                                    "outs": [],
                                    "name": f"{inst['name']}-sw{counter}",
                                    "opcode": "NoOp",
                                    "sync_info": {
                                        "on_wait": [w],
                                        "on_update": [],
                                    },
                                }
                            )
                        si["on_wait"] = [waits[-1]]
                out.append(inst)
            blk["instructions"] = out
    return json.dumps(m).encode()


def _patch_compile():
    """Route every BIR compile through _split_sync_waits."""
    global _PATCHED
    if _PATCHED:
        return
    _PATCHED = True
    import concourse.bass_utils as bu
    import concourse.bass2jax as b2j

    orig = bu.compile_bir_kernel

    def patched(bir_json, tmpdir, neff_name="file.neff"):
        return orig(_split_sync_waits(bir_json), tmpdir, neff_name)

    bu.compile_bir_kernel = patched
    b2j.compile_bir_kernel = patched


def build_bass(mm_dtype="fp32"):
    """Build the per-core Bass module. mm_dtype: fp32 | fp32r | fp16x2.

    fp32   — exact, but PE runs fp32 matmuls at 4 cycles/row.
    fp32r  — 1 cycle/row, TF32-like precision (~2e-4 rel err).
    fp16x2 — g split host-side into fp16 hi+lo planes (same total bytes as
             f32), recal split on-chip; 2 streaming passes at 1 cycle/row
             reach ~fp32 precision: pooled = [rhi|rlo]@ghi + rhi@glo, all
             accumulated in one PSUM group (dropped rlo*glo term ~2^-22).
    """
    _patch_compile()
    nc = bass.Bass()
    F16 = mybir.dt.float16
    split16 = mm_dtype == "fp16x2"
    half16 = mm_dtype == "fp16"
    # float32r is bit-identical to f32 in memory; typing the operand chain as
    # f32r lets the PE run matmuls at 1 cycle/row instead of fp32's 4.
    MD = mybir.dt.float32r if mm_dtype == "fp32r" else F32
    # lmk host-transposed to [p, t, (n b), k]: softmax runs directly on it,
    # no on-chip transpose needed; rows are 2.9 KB contiguous.
    lmk = nc.dram_tensor("lmk", [P, T, NB * K], F32, kind="ExternalInput")
    # gmap host layout: hw split as (t p), stored [n, b, p, t, c] so each SBUF
    # partition row is one contiguous run (T*C elements) per (n, b) DMA.
    if split16:
        # hi and lo planes packed per partition row: one 14 KB contiguous run
        g2_d = nc.dram_tensor("g2", [N, BL, P, 2, T, C], F16, kind="ExternalInput")
    elif half16:
        gh_d = nc.dram_tensor("gh", [N, BL, P, T, C], F16, kind="ExternalInput")
    else:
        gmap = nc.dram_tensor("gmap", [N, BL, P, T, C], MD, kind="ExternalInput")
    out_d = nc.dram_tensor("out", [BL, K, C], F32, kind="ExternalOutput")

    with tile.TileContext(nc) as tc:
        with (
            tc.tile_pool(name="consts", bufs=1) as consts,
            tc.tile_pool(name="lmp", bufs=1) as lmp,
            tc.tile_pool(name="recp", bufs=1) as recp,
            tc.tile_pool(name="gp", bufs=4) as gp,
            tc.tile_pool(name="small", bufs=8) as small,
            tc.tile_pool(name="outp", bufs=1) as outp,
            tc.tile_pool(name="pmp", bufs=4, space="PSUM") as pmp,
        ):
            sb_eps = consts.tile([K, 1], F32)
            nc.vector.memset(sb_eps, EPS)

            sb_lmk = lmp.tile([P, T, NB * K], F32)
            nc.sync.dma_start(out=sb_lmk, in_=lmk[:, :, :])

            # recal[p_hw, t, (n b k)] after softmax over k
            sb_rec = recp.tile([P, T, NB * K], MD)
            sb_rcp = recp.tile([P, T, NB], F32)
            if split16:
                # hi/lo packed per nb with lo at column 32 so the PSUM lo rows
                # start at partition 32 (engine partition base must be 0/32/64/96);
                # columns 13..31 stay zero.
                sb_rhl = recp.tile([P, T, NB, 45], F16)
                nc.vector.memset(sb_rhl[:, :, :, K:32], 0.0)
            elif half16:
                sb_rh = recp.tile([P, T, NB * K], F16)

            for t in range(T):
                rec_t = sb_rec[:, t, :]
                nc.scalar.activation(rec_t, sb_lmk[:, t, :], AF.Exp)
                rec3 = rec_t.rearrange("p (nb k) -> p nb k", k=K)
                ssum = small.tile([P, NB], F32, tag="ssum")
                nc.vector.reduce_sum(ssum, rec3, axis=mybir.AxisListType.X)
                nc.vector.reciprocal(sb_rcp[:, t, :], ssum)
                nc.vector.tensor_mul(
                    rec3,
                    rec3,
                    sb_rcp[:, t, :].unsqueeze(2).broadcast_to([P, NB, K]),
                )
                if split16:
                    rhl4 = sb_rhl[:, t]
                    nc.vector.tensor_copy(rhl4[:, :, 0:K], rec3)
                    nc.vector.tensor_sub(rhl4[:, :, 32 : 32 + K], rec3, rhl4[:, :, 0:K])
                elif half16:
                    nc.vector.tensor_copy(sb_rh[:, t, :], rec_t)

            sb_out = outp.tile([K, BL, C], F32)
            for b in range(BL):
                if split16:
                    ghi_tiles, glo_tiles = [], []
                    for n in range(N):
                        sb_g2 = gp.tile([P, 2, T, C], F16, tag="g")
                        nc.sync.dma_start(out=sb_g2, in_=g2_d[n, b])
                        ghi_tiles.append(sb_g2[:, 0])
                        glo_tiles.append(sb_g2[:, 1])
                    pm = pmp.tile([32 + K, C], F32, tag="pm")
                    for n in range(N):
                        nb = n * BL + b
                        for t in range(T):
                            nc.tensor.matmul(
                                pm,
                                sb_rhl[:, t, nb, :],
                                ghi_tiles[n][:, t, :],
                                start=(n == 0 and t == 0),
                                stop=False,
                            )
                    for n in range(N):
                        nb = n * BL + b
                        for t in range(T):
                            nc.tensor.matmul(
                                pm[0:K, :],
                                sb_rhl[:, t, nb, 0:K],
                                glo_tiles[n][:, t, :],
                                start=False,
                                stop=(n == N - 1 and t == T - 1),
                            )
                    # pooled = pm[hi rows] + pm[lo-correction rows]
                    tmp = small.tile([K, C], F32, tag="tmp")
                    nc.scalar.copy(tmp, pm[32 : 32 + K, :])
                    pooled = small.tile([K, C], F32, tag="pooled")
                    nc.vector.tensor_add(pooled, tmp, pm[0:K, :])
                else:
                    g_tiles = []
                    for n in range(N):
                        sb_g = gp.tile([P, T, C], F16 if half16 else MD, tag="g")
                        nc.sync.dma_start(
                            out=sb_g, in_=gh_d[n, b] if half16 else gmap[n, b]
                        )
                        g_tiles.append(sb_g)
                    pm = pmp.tile([K, C], F32, tag="pm")
                    for n in range(N):
                        nb = n * BL + b
                        for t in range(T):
                            nc.tensor.matmul(
                                pm,
                                sb_rh[:, t, nb * K : (nb + 1) * K]
                                if half16
                                else sb_rec[:, t, nb * K : (nb + 1) * K],
                                g_tiles[n][:, t, :],
                                start=(n == 0 and t == 0),
                                stop=(n == N - 1 and t == T - 1),
                            )
                    pooled = pm
                # L2 normalize over C
                sq = small.tile([K, C], F32, tag="sq")
                ss = small.tile([K, 1], F32, tag="ss")
                nc.scalar.activation(sq, pooled, AF.Square, accum_out=ss)
                nrm = small.tile([K, 1], F32, tag="nrm")
                nc.scalar.activation(nrm, ss, AF.Sqrt)
                nc.vector.tensor_max(nrm, nrm, sb_eps)
                rcpn = small.tile([K, 1], F32, tag="rcpn")
                nc.vector.reciprocal(rcpn, nrm)
                nc.vector.tensor_scalar_mul(sb_out[:, b, :], in0=pooled, scalar1=rcpn)
                # SWDGE: keeps the HWDGE prefetch ring free of dependent DMAs
                nc.gpsimd.dma_start(out=out_d[b], in_=sb_out[:, b, :])
    return nc


def _prep_in_maps(local_landmarks, global_maps, mm_dtype="fp32"):
    l = np.asarray(local_landmarks, dtype=np.float32).reshape(N, B, K, T, P)
    # [P, T, N, B, K]; hw = t*P + p
    lt = np.ascontiguousarray(l.transpose(4, 3, 0, 1, 2))
    g = np.asarray(global_maps, dtype=np.float32).reshape(N, B, C, T, P)
    # [N, B, P, T, C]: hw = t*P + p; partition rows contiguous per (n, b).
    gt = np.ascontiguousarray(g.transpose(0, 1, 4, 3, 2))
    if mm_dtype == "fp16x2":
        ghi = gt.astype(np.float16)
        glo = (gt - ghi.astype(np.float32)).astype(np.float16)
        # [N, B, P, 2, T, C]
        g2 = np.stack([ghi, glo], axis=3)
    elif mm_dtype == "fp16":
        gh = gt.astype(np.float16)
    in_maps = []
    for c in range(N_CORES):
        bs = slice(BL * c, BL * (c + 1))
        m = {"lmk": np.ascontiguousarray(lt[:, :, :, bs]).reshape(P, T, NB * K)}
        if mm_dtype == "fp16x2":
            m["g2"] = np.ascontiguousarray(g2[:, bs])
        elif mm_dtype == "fp16":
            m["gh"] = np.ascontiguousarray(gh[:, bs])
        else:
            m["gmap"] = np.ascontiguousarray(gt[:, bs])
        in_maps.append(m)
    return in_maps


def run_on_cores(local_landmarks, global_maps, trace=False, mm_dtype="fp32"):
    """Returns (full_output [32,13,512], BassKernelResults)."""
    from concourse.bass_utils import run_bass_kernel_spmd

    nc = build_bass(mm_dtype=mm_dtype)
    in_maps = _prep_in_maps(local_landmarks, global_maps, mm_dtype=mm_dtype)
    res = run_bass_kernel_spmd(
        nc, in_maps, core_ids=list(range(N_CORES)), trace=trace
    )
    out = np.concatenate([r["out"] for r in res.results], axis=0)
    return np.ascontiguousarray(out, dtype=np.float32), res


def kernel(local_landmarks, global_maps):
    out, _ = run_on_cores(local_landmarks, global_maps, trace=False)
    return out
